# revision 47
# baseline (speedup 1.0000x reference)
"""Trainium2 Bass kernel for nn_DeepVoxels (octree prune + mean-pool round trip).

Self-contained: takes full octree [1, 64, 299592] f32, shards 8 features per
NeuronCore across 8 cores, returns full [1, 64, 299592] f32.

Closed form of the reference (derived analytically, numerically verified):
  keep-mask cascade over 5 levels (parents at v[dl:2dl], quirky child map):
    k_i[j] = (v[dl_i+j] >= EPS) * k_parent, parent = prev-level tail (j<start)
             or same-level head block j//8 (j>=start)
  out[7607:8192)    = k3means tail = mean-8 of k4means[0:4680)
  out[32768:65536)  = k4means = mean-8 of (leaf * EM)
  out[65536:299592) = (leaf * rep8(keep4))[28088:]
  everything else 0, where EM[m] = keep4[4680+m] for m<28088 (identity:
  keep4[4680+m] = (leaf[m]>=EPS)*keep4[m//8]), EM[m] = rep8(keep4)[m] else.

Synchronization notes (hard-won):
  * raw bass: DVE->DVE RAW needs drain() between ops; DVE->DMA needs
    drain().then_inc(sem) (then_inc on a compute op fires at retire, before
    SBUF writes commit).
  * DMA completion counting on a shared semaphore is only sound when the
    wait covers ALL DMAs issued on that semaphore so far and no later DMA
    on the same semaphore is already in flight (SDMA engines process ring
    slices independently, so partial counts can be satisfied by later
    DMAs' increments). Hence dedicated per-stream / per-buffer-parity sems.

Perf notes (cost model single-shot 91.9us -> 77.4us, marginal 88.6us ->
69.1us this round; HW chain-slope samples through the noisy axon dispatch
path: baseline 103.8us -> 73.9-89us, converging toward the model marginal
as sample count grows):
  * tailB (keep4[8192:32768]) runs at full partition width as [128, 1536]
    (W, f-major rows 16f+i) instead of [24, 8192]: 5.3x fewer DVE cycles
    on the serial cascade path.
  * keep4 never round-trips through DRAM: Kb[f] is assembled SBUF->SBUF
    from k4c (chunk 0) + W rows 12f:12f+12, all on HWDGE rings (SWDGE
    issue on Pool costs ~1us/DMA vs ~0.6us HWDGE, and serializes).
  * TensorScalarPtr/STT is NOT supported on the Pool engine by the V3 ISA
    (walrus rejects it) even though bass/cost-model accept it — the head
    fixes must stay on DVE. Plain TensorTensor DOES work on Pool (HW
    verified), so the mean-8 reductions run there as a 4+2+1 pairwise
    tree of lane adds (raw sums; ACT scales by 1/8 and 1/64 via
    activation-mul). Drain/op overhead on the Q7 is LARGE on real HW
    (~0.6us each): the drain-chained in-place accumulate (7 drains) cost
    ~12us/rep more (HW chain-slope 96.9us), a 3-op stride-2 tree was
    also slower (88.4us) — this 7-op/3-drain tree measures 83.3-84.8us.
  * Signal split per feature: prodE fires after the product (gates
    E-stores + Kb reloads + Lb rows 14:128 reload — the head fixes only
    touch rows 0:14), prod fires after the fixes (gates Lb rows 0:14
    reload + Pool lane-adds).
  * L/E buffers are 4-deep so features 0-3 prefetch during the cascade,
    filling the DMA idle window; steady-state is DMA-paced (~6.3us/feat).
  * headA is split at 3584 so the hs2 parent-scatter round-trip hides
    under A2+tailA; feature 7's reduce+scales run on DVE (idle at the
    tail) instead of Pool/ACT to shorten the last-feature chain.
  * A wide [32,1024]-partition level-4 cascade (2.3us of DVE vs 8.6us)
    was tried and REVERTED: the 6+ parent-gather/bounce DMAs it needs
    serialize on HWDGE issue (~0.6us each) and on the DMA engines behind
    bulk prefetches, netting +5us total.
"""
import sys

sys.path.insert(0, "/opt/trn_rl_repo")

import numpy as np

OCT = 299592
F = 64
FEATS = 8
N_CORES = 8
EPS = 1e-5

_cache = {}


def _build(reps: int = 1):
    import concourse.bass as bass
    import concourse.mybir as mybir

    F32 = mybir.dt.float32
    ge, mul = mybir.AluOpType.is_ge, mybir.AluOpType.mult
    add = mybir.AluOpType.add
    X = mybir.AxisListType.X

    nc = bass.Bass()
    x = nc.dram_tensor("x", [FEATS, OCT], F32, kind="ExternalInput")
    y = nc.dram_tensor("y", [FEATS, OCT], F32, kind="ExternalOutput")

    vc = {"n": 0}
    V = {}

    def vsig(v, name, vs):
        vc["n"] += 1
        V[name] = vc["n"]
        return v.drain().then_inc(vs, 1)

    from contextlib import ExitStack

    with ExitStack() as ctx:
        sm = ctx.enter_context(nc.sbuf_tensor([8, 8192], F32))
        k4c = ctx.enter_context(nc.sbuf_tensor([8, 8192], F32))
        W = ctx.enter_context(nc.sbuf_tensor([128, 1536], F32))
        hs2 = ctx.enter_context(nc.sbuf_tensor([128, 192], F32))
        L0 = ctx.enter_context(nc.sbuf_tensor([128, 2048], F32))
        L1 = ctx.enter_context(nc.sbuf_tensor([128, 2048], F32))
        L2 = ctx.enter_context(nc.sbuf_tensor([128, 2048], F32))
        L3 = ctx.enter_context(nc.sbuf_tensor([128, 2048], F32))
        E0 = ctx.enter_context(nc.sbuf_tensor([128, 2048], F32))
        E1 = ctx.enter_context(nc.sbuf_tensor([128, 2048], F32))
        E2 = ctx.enter_context(nc.sbuf_tensor([128, 2048], F32))
        E3 = ctx.enter_context(nc.sbuf_tensor([128, 2048], F32))
        K0 = ctx.enter_context(nc.sbuf_tensor([128, 256], F32))
        K1 = ctx.enter_context(nc.sbuf_tensor([128, 256], F32))
        M0 = ctx.enter_context(nc.sbuf_tensor([128, 256], F32))
        M1 = ctx.enter_context(nc.sbuf_tensor([128, 256], F32))
        T4 = ctx.enter_context(nc.sbuf_tensor([128, 1024], F32))
        MS0 = ctx.enter_context(nc.sbuf_tensor([128, 256], F32))
        MS1 = ctx.enter_context(nc.sbuf_tensor([128, 256], F32))
        J0 = ctx.enter_context(nc.sbuf_tensor([19, 32], F32))
        J1 = ctx.enter_context(nc.sbuf_tensor([19, 32], F32))
        JS0 = ctx.enter_context(nc.sbuf_tensor([19, 32], F32))
        JS1 = ctx.enter_context(nc.sbuf_tensor([19, 32], F32))
        s_init = ctx.enter_context(nc.semaphore("s_init"))
        s_sm = ctx.enter_context(nc.semaphore("s_sm"))
        s_load0 = ctx.enter_context(nc.semaphore("s_load0"))
        s_load1 = ctx.enter_context(nc.semaphore("s_load1"))
        s_store0 = ctx.enter_context(nc.semaphore("s_store0"))
        s_store1 = ctx.enter_context(nc.semaphore("s_store1"))
        s_store2 = ctx.enter_context(nc.semaphore("s_store2"))
        s_store3 = ctx.enter_context(nc.semaphore("s_store3"))
        g_hs = ctx.enter_context(nc.semaphore("g_hs"))
        g_k40 = ctx.enter_context(nc.semaphore("g_k40"))
        g_k41 = ctx.enter_context(nc.semaphore("g_k41"))
        a_mj0 = ctx.enter_context(nc.semaphore("a_mj0"))
        a_mj1 = ctx.enter_context(nc.semaphore("a_mj1"))
        vs = ctx.enter_context(nc.semaphore("vs"))
        ps = ctx.enter_context(nc.semaphore("ps"))
        fin = ctx.enter_context(nc.semaphore("fin"))
        # Pool issues no DMAs anymore (Kb copies live on HWDGE rings), so
        # skip its expensive dge_drain at block exit; the fin protocol
        # already guarantees all engines' work completed
        block = ctx.enter_context(nc.Block(no_gpsimd_drain=True))
        Lb, Eb, Kb, Mb, Jb = [L0, L1, L2, L3], [E0, E1, E2, E3], [K0, K1], [M0, M1], [J0, J1]
        MSb, JSb = [MS0, MS1], [JS0, JS1]
        s_load = [s_load0, s_load1]
        s_store = [s_store0, s_store1, s_store2, s_store3]
        SST = [64, 64, 64, 64]  # per-rep store-count totals per Eb buffer
        g_k4 = [g_k40, g_k41]
        a_mj = [a_mj0, a_mj1]

        # per-rep semaphore totals (for the timing variant reps>1)
        S_INIT_T, SLOAD_T, SSTORE_T = 32, 128, 128
        S_SM_T = 16
        G_HS_T, G_K4_T, A_MJ_T = 16, 128, 192
        PS_T = 8  # Pool lane-add completions (one per feature)

        def rep8(src2d, n):
            return src2d[:, :, None].to_broadcast([src2d.shape[0], n, 8])

        def blk(ap2d, n):
            return ap2d.rearrange("p (t e) -> p t e", e=8)

        # ---------------- vector program ----------------
        @block.vector
        def _(v):
          for r in range(reps):
            v.wait_ge(s_sm, S_SM_T * (r + 1))  # sm loaded (levels 0-3 only)
            # ---- keep-mask cascade (strict chain: drain between ops) ----
            v.tensor_scalar(sm[:, 8:16], sm[:, 8:16], EPS, None, ge)
            v.drain()
            v.tensor_tensor(sm[:, 9:16], sm[:, 9:16], sm[:, 8:9].to_broadcast([8, 7]), mul)
            v.drain()
            v.scalar_tensor_tensor(sm[:, 64:72], sm[:, 64:72], EPS,
                                   sm[:, 15:16].to_broadcast([8, 8]), ge, mul)
            v.drain()
            v.scalar_tensor_tensor(blk(sm[:, 72:128], 7), blk(sm[:, 72:128], 7), EPS,
                                   rep8(sm[:, 64:71], 7), ge, mul)
            v.drain()
            v.scalar_tensor_tensor(blk(sm[:, 512:584], 9), blk(sm[:, 512:584], 9), EPS,
                                   rep8(sm[:, 119:128], 9), ge, mul)
            v.drain()
            v.scalar_tensor_tensor(blk(sm[:, 584:1024], 55), blk(sm[:, 584:1024], 55), EPS,
                                   rep8(sm[:, 512:567], 55), ge, mul)
            v.drain()
            v.scalar_tensor_tensor(blk(sm[:, 4096:4680], 73), blk(sm[:, 4096:4680], 73), EPS,
                                   rep8(sm[:, 951:1024], 73), ge, mul)
            v.drain()
            v.scalar_tensor_tensor(blk(sm[:, 4680:8192], 439), blk(sm[:, 4680:8192], 439), EPS,
                                   rep8(sm[:, 4096:4535], 439), ge, mul)
            v.drain()
            v.wait_ge(s_init, S_INIT_T * (r + 1))  # P4 loaded
            # headA split at 3584: the hs2 scatter only reads [439:3511), so
            # signal after A1 and hide the scatter round-trip under A2+tailA
            v.scalar_tensor_tensor(blk(k4c[0:8, 0:3584], 448), blk(k4c[0:8, 0:3584], 448), EPS,
                                   rep8(sm[:, 7607:8055], 448), ge, mul)
            vsig(v, f"headA_{r}", vs)
            v.scalar_tensor_tensor(blk(k4c[0:8, 3584:4680], 137), blk(k4c[0:8, 3584:4680], 137), EPS,
                                   rep8(sm[:, 8055:8192], 137), ge, mul)
            v.scalar_tensor_tensor(blk(k4c[0:8, 4680:8192], 439),
                                   blk(k4c[0:8, 4680:8192], 439), EPS,
                                   rep8(k4c[0:8, 0:439], 439), ge, mul)
            vsig(v, f"tailA_{r}", vs)
            v.wait_ge(g_hs, G_HS_T * (r + 1))
            # tailB widened to [128,1536] (full partition width): W row 16f+i
            # holds v4[f][8192+1536i : +1536]; parents hs2 same row =
            # keep4[f][439+192i : +192]
            v.scalar_tensor_tensor(blk(W[:, :], 192), blk(W[:, :], 192), EPS,
                                   rep8(hs2[:, :], 192), ge, mul)
            vsig(v, f"tailB_{r}", vs)

            # ---- leaf phase: product into Eb[g%3]; head (elems <28088) fixed
            # via two base-0 rectangles [0:14)x[0:1464) u [0:13)x[1464:2048);
            # raw leaves stay in Lb so (L>=EPS) supplies the element mask.
            # Mean-8 reduces run on the Pool engine (7 strided lane adds);
            # M/J scaling runs on ACT; DVE keeps only product+fixes+J-reduce ----
            def jred(g, skip_ps=False):
                # second-stage J reduce for feature g (raw sum-64 from raw Mb)
                bj = g % 2
                if not skip_ps:
                    v.wait_ge(ps, PS_T * r + g + 1)
                # Jb[bj] reuse: ACT's JbS mul of g-2 must be done (implied by
                # its store completions on a_mj)
                amt = A_MJ_T * r + (48 * ((g - 2) // 2 + 1) if g >= 2 else 0)
                if amt > 0:
                    v.wait_ge(a_mj[bj], amt)
                v.tensor_reduce(Jb[bj][:, :], blk(Mb[bj][0:19, :], 32), X, add)
                vsig(v, f"jred{g}_{r}", vs)

            for g in range(FEATS):
                    b = g % 2
                    q = g % 4
                    e = g % 4
                    v.wait_ge(g_k4[b], G_K4_T * r + 32 * (g // 2 + 1))
                    v.wait_ge(s_load[b], SLOAD_T * r + 32 * (g // 2 + 1))
                    # Eb[e] reuse: stores AND Pool lane-adds of the prior user
                    # (absolute feature index 8r+g-4) must be done
                    base = SST[e] * r
                    amt = base + (32 * ((g - 4) // 4 + 1) if g >= 4 else 0)
                    if amt > 0:
                        v.wait_ge(s_store[e], amt)
                    prior = 8 * r + g - 4
                    if prior >= 0:
                        v.wait_ge(ps, prior + 1)
                    v.tensor_tensor(blk(Eb[e][:, :], 256), blk(Lb[q][:, :], 256),
                                    rep8(Kb[b][:, :], 256), mul)
                    # the head fixes only touch rows 0:14 / m < 28088; the
                    # E-stores read row 13 col 1464: onward, so they (and the
                    # Kb reloads) can go as soon as the product retires
                    vsig(v, f"prodE{g}_{r}", vs)
                    v.scalar_tensor_tensor(Eb[e][0:14, 0:1464], Lb[q][0:14, 0:1464], EPS,
                                           Eb[e][0:14, 0:1464], ge, mul)
                    v.scalar_tensor_tensor(Eb[e][0:13, 1464:2048], Lb[q][0:13, 1464:2048], EPS,
                                           Eb[e][0:13, 1464:2048], ge, mul)
                    # the Lb reload and the Pool lane-adds need the fixes too
                    vsig(v, f"prod{g}_{r}", vs)
                    if g >= 1 and g < FEATS - 1:
                        jred(g - 1)
                    if g == FEATS - 1:
                        # last feature: reduce on DVE (Pool's lane-adds would
                        # sit on the critical-path tail; DVE is idle here).
                        # jred(6) first: its Mb[0] input is ready and ACT f6
                        # is waiting on it
                        jred(g - 1)
                        # Mb[b] reuse: ACT's MbS mul of g-2 must have read it
                        v.wait_ge(a_mj[b], A_MJ_T * r + 48 * ((g - 2) // 2 + 1))
                        v.tensor_reduce(Mb[b][:, :], blk(Eb[e][:, :], 256), X, add)
                        vsig(v, f"mred{g}_{r}", vs)
            jred(FEATS - 1, skip_ps=True)
            # f7's scaling on DVE as well: ACT would serialize it behind f6's
            # store issues. MSb/JSb reuse guards are the f5 store completions
            v.tensor_scalar(MSb[1][:, :], Mb[1][:, :], 0.125, None, mul)
            v.tensor_scalar(JSb[1][:, :], Jb[1][:, :], 0.015625, None, mul)
            vsig(v, f"scale7_{r}", vs)
          v.wait_ge(fin, 3)
          for s in (s_init, s_sm, s_load0, s_load1, s_store0, s_store1, s_store2,
                    s_store3, g_hs, g_k40, g_k41, a_mj0, a_mj1, vs, ps, fin):
              v.sem_clear(s)

        # ---------------- sync ring: bulk loads + leaf stores ----------------
        @block.sync
        def _(sync):
          for r in range(reps):
            if r > 0:
                # sm/k4c/W reload hazards: all of the prior rep's Kb copies
                # (read k4c+W) and the hs2 gather (reads k4c) must be done
                sync.wait_ge(g_k4[0], G_K4_T * r)
                sync.wait_ge(g_k4[1], G_K4_T * r)
                sync.wait_ge(g_hs, G_HS_T * r)
            sync.dma_start(sm[:, :], x[:, 0:8192]).then_inc(s_sm, 16)
            sync.dma_start(k4c[0:8, :], x[:, 32768:40960]).then_inc(s_init, 16)
            # W row 16f+i = v4[f][8192 + 1536i : +1536], one DMA
            p4b = x[:, 40960:65536].rearrange("f (i w) -> f i w", w=1536)
            sync.dma_start(W[:, :], p4b).then_inc(s_init, 16)
            for f in range(4):
                if r > 0:
                    # Lb[f] last read by the product/head-fix of (r-1, f+4)
                    sync.wait_ge(vs, V[f"prod{f + 4}_{r - 1}"])
                sync.dma_start(Lb[f][14:128, :], x[f, 66120:299592]).then_inc(s_load[f % 2], 16)
                sync.dma_start(Lb[f][0:14, :], x[f, 37448:66120]).then_inc(s_load[f % 2], 16)
            # chunk 0 of keep4[f] ([0:8192)) is final after tailA; the r>0
            # overwrite hazard on Kb[f<2] is covered by the Lb waits above
            sync.wait_ge(vs, V[f"tailA_{r}"])
            for f in range(2):
                sync.dma_start(Kb[f][0:32, :], k4c[f : f + 1, :]).then_inc(g_k4[f], 16)
            sync.wait_ge(vs, V[f"tailB_{r}"])
            for f in range(2):
                sync.dma_start(
                    Kb[f][32:128, :], W[16 * f : 16 * f + 16, :]
                ).then_inc(g_k4[f], 16)
            for f in range(FEATS):
                b = f % 2
                e = f % 4
                # the E-stores read row 13 col 1464: onward, final at prodE;
                # Kb[b] chunk 0 for f+2 is free once product f read it
                sync.wait_ge(vs, V[f"prodE{f}_{r}"])
                sync.dma_start(y[f, 65536:66120], Eb[e][13:14, 1464:2048]).then_inc(s_store[e], 16)
                sync.dma_start(y[f, 66120:299592], Eb[e][14:128, :]).then_inc(s_store[e], 16)
                if f + 2 < FEATS:
                    sync.dma_start(Kb[b][0:32, :], k4c[f + 2 : f + 3, :]).then_inc(g_k4[b], 16)
                    sync.dma_start(
                        Kb[b][32:128, :], W[16 * (f + 2) : 16 * (f + 2) + 16, :]
                    ).then_inc(g_k4[b], 16)
                if f + 4 < FEATS:
                    # Lb rows 14:128 are free at prodE (only the product reads
                    # them); rows 0:14 are still read by the head fixes
                    sync.dma_start(Lb[f % 4][14:128, :], x[f + 4, 66120:299592]).then_inc(s_load[f % 2], 16)
                    sync.wait_ge(vs, V[f"prod{f}_{r}"])
                    sync.dma_start(Lb[f % 4][0:14, :], x[f + 4, 37448:66120]).then_inc(s_load[f % 2], 16)
          for e in range(4):
              sync.wait_ge(s_store[e], SST[e] * reps)
          sync.sem_inc(fin, 1)

        # ---- pool ring: keep4 chunks 1-3 straight from W (no DRAM round
        # trip): Kb[f][32:128] <- W[12f:12f+12] after tailB / prod(f-2),
        # PLUS the mean-8 reductions as 7 strided lane adds per feature
        # (raw sums; ACT scales them). The next-rep W load on the sync ring
        # cannot race the Kb reads: it is issued only after all of this
        # rep's stores completed ----
        @block.gpsimd
        def _(gpsimd):
          for r in range(reps):
            for f in range(FEATS):
                b = f % 2
                e = f % 4
                if f == FEATS - 1:
                    # DVE did this feature's reduce; just forward the signal
                    gpsimd.wait_ge(vs, V[f"mred{f}_{r}"])
                    gpsimd.sem_inc(ps, 1)
                    continue
                gpsimd.wait_ge(vs, V[f"prod{f}_{r}"])
                # Mb[b] reuse: DVE's jred{f-2} and ACT's MbS mul{f-2} (implied
                # by its store completions) must be done
                if f >= 2:
                    gpsimd.wait_ge(vs, V[f"jred{f - 2}_{r}"])
                    gpsimd.wait_ge(a_mj[b], A_MJ_T * r + 48 * ((f - 2) // 2 + 1))
                # mean-8 numerator as a 3-level pairwise tree: the four
                # leaf-pair sums are independent (no drains between), as are
                # the two second-level sums; T4 is Pool-private scratch
                lanes = blk(Eb[e][:, :], 256)
                for q in range(4):
                    gpsimd.tensor_tensor(T4[:, 256 * q : 256 * (q + 1)],
                                         lanes[:, :, 2 * q], lanes[:, :, 2 * q + 1], add)
                gpsimd.drain()
                gpsimd.tensor_tensor(T4[:, 0:256], T4[:, 0:256], T4[:, 256:512], add)
                gpsimd.tensor_tensor(T4[:, 512:768], T4[:, 512:768], T4[:, 768:1024], add)
                gpsimd.drain()
                gpsimd.tensor_tensor(Mb[b][:, :], T4[:, 0:256], T4[:, 512:768], add)
                gpsimd.drain().then_inc(ps, 1)
          gpsimd.wait_ge(g_k4[0], G_K4_T * reps)
          gpsimd.wait_ge(g_k4[1], G_K4_T * reps)
          gpsimd.sem_inc(fin, 1)

        # ------------- act ring: M/J scaling + means/k3means stores -------------
        @block.scalar
        def _(scalar):
          # zero regions of y are covered by the donated zero-initialized
          # output buffers (run_bass_via_pjrt / run_bass_kernel_spmd both
          # pre-zero ExternalOutputs)
          for r in range(reps):
            scalar.wait_ge(vs, V[f"headA_{r}"])
            # hs2 row 16f+i = keep4[f][439+192i : +192] (tailB parents)
            hsrc = k4c[0:8, 439:3511].rearrange("p (i t) -> p i t", t=192)
            scalar.dma_start(hs2[:, :], hsrc).then_inc(g_hs, 16)
            for f in range(FEATS):
                b = f % 2
                if f == FEATS - 1:
                    # DVE scaled this one; just store
                    scalar.wait_ge(vs, V[f"scale7_{r}"])
                else:
                    # raw Mb ready (Pool lane-adds done)
                    scalar.wait_ge(ps, PS_T * r + f + 1)
                    # MSb/JSb reuse: prior stores (f-2) must have drained
                    if f >= 2:
                        scalar.wait_ge(a_mj[b], A_MJ_T * r + 48 * ((f - 2) // 2 + 1))
                    scalar.mul(MSb[b][:, :], Mb[b][:, :], 0.125)
                    scalar.wait_ge(vs, V[f"jred{f}_{r}"])
                    # J from unscaled sum-64: x0.015625 = 1/64 (exact)
                    scalar.mul(JSb[b][:, :], Jb[b][:, :], 0.015625)
                    scalar.drain()
                scalar.dma_start(y[f, 32768:65536], MSb[b][:, :]).then_inc(a_mj[b], 16)
                scalar.dma_start(y[f, 7607:8183], JSb[b][0:18, :]).then_inc(a_mj[b], 16)
                scalar.dma_start(y[f, 8183:8192], JSb[b][18:19, 0:9]).then_inc(a_mj[b], 16)
            scalar.wait_ge(a_mj[0], A_MJ_T * (r + 1))
            scalar.wait_ge(a_mj[1], A_MJ_T * (r + 1))
          scalar.sem_inc(fin, 1)

    return nc


def kernel(octree: np.ndarray) -> np.ndarray:
    from concourse.bass_utils import run_bass_kernel_spmd

    octree = np.ascontiguousarray(octree, dtype=np.float32)
    assert octree.shape == (1, F, OCT)

    if "nc" not in _cache:
        _cache["nc"] = _build()
    nc = _cache["nc"]

    in_maps = [
        {"x": octree[0, c * FEATS : (c + 1) * FEATS, :]} for c in range(N_CORES)
    ]
    res = run_bass_kernel_spmd(nc, in_maps, core_ids=list(range(N_CORES)))
    _cache["last"] = res
    out = np.empty((1, F, OCT), np.float32)
    for c in range(N_CORES):
        out[0, c * FEATS : (c + 1) * FEATS, :] = res.results[c]["y"]
    return out

